# revision 30
# baseline (speedup 1.0000x reference)
"""Mixture-of-Experts Trainium2 kernel (8-core SPMD, token-sharded, bf16).

v6: head restructure guided by the NTFF traces of v4 (470.2us) and the
failed v5 (478.9us):
  * v4's PE only went dense at ~25us: the gpsimd queue ran ahead and
    issued 8MB of next-expert prefetches whose SDMA packets round-robin-
    stole HBM bandwidth from the critical xt loads.
  * v5 split w1_0 into 16 small SWDGE DMAs -- issue-rate limited (~0.9us
    per issue, stretching to 3us+ under descriptor-ring backlog), which
    starved L1 even harder.
  * v6: 4 xt tiles [P,DC,512] and 4 w1_0 fc-group tiles [P,4,DC*128],
    each host-packed so a tile is 4KB-contiguous per partition (128 fat
    descriptors per DMA).  Critical tiles go first on each ring (sync:
    xt0,xt2; scalar: xt1,xt3; gpsimd: w1g0-3); big prefetches are emitted
    behind them so ring FIFO keeps the critical window clean.  gw/b1 ride
    the otherwise-idle vector queue.  Warmup extended to 96 matmuls to
    keep the HAM clock-gate warm until the first real matmul (~10.5us).
  * tail kept from v4 (measured at its floor: ~0.75us combine + 0.6us
    issue + ~3us DMA completion receipt); output DMAs alternate rings.
"""

import numpy as np
import ml_dtypes
from contextlib import ExitStack

import bass_rust as _bass_rust
import concourse.bass as bass
import concourse.mybir as mybir
import concourse.tile as tile
from concourse.bass_utils import run_bass_kernel_spmd

BF16 = mybir.dt.bfloat16
F32 = mybir.dt.float32
N_CORES = 8
P = 128


# ---------------------------------------------------------------------------
# Workaround for walrus "Too many sync wait commands" (see baseline).
# ---------------------------------------------------------------------------
_split_ctr = [0]


def _split_multi_waits(nc):
    for f in nc.m.functions:
        for blk in f.blocks:
            insts = blk.instructions
            i = 0
            while i < len(insts):
                inst = insts[i]
                si = getattr(inst, "sync_info", None)
                waits = list(si.on_wait) if si is not None and si.on_wait else []
                if len(waits) > 1:
                    si.on_wait = waits[-1:]
                    for w in waits[:-1]:
                        _split_ctr[0] += 1
                        ev = mybir.InstEventSemaphore(
                            name=f"I-wsplit-{_split_ctr[0]}", ins=[], outs=[]
                        )
                        ev.engine = inst.engine
                        ev.sync_info = _bass_rust.SyncInfo(
                            on_wait=[w], on_update=[]
                        )
                        insts.insert(i, ev)
                        i += 1
                i += 1


# ---------------------------------------------------------------------------
# Device kernel
# ---------------------------------------------------------------------------
def build_moe_kernel(K: int, T: int, D: int, DF: int):
    assert T % 512 == 0 and D % P == 0 and DF % P == 0
    TT = T // P       # 128-token tiles
    TC = T // 512     # 512-token chunks
    DC = D // P       # D chunks of 128
    FC = DF // P      # F chunks of 128
    FG = 4            # fc per w1_0 head tile
    NG = FC // FG

    nc = bass.Bass("TRN2", target_bir_lowering=False)

    # x packed chunk-major so each xt tile is one CONTIGUOUS 512KB block
    # (4KB reads strided 16KB measured only ~107GB/s -- 25% HBM row use):
    # xq[tcc, p, dc, j] = x[tcc*512+j, dc*128+p]
    xq = nc.declare_dram_parameter("xq", [TC, P, DC, 512], BF16, isOutput=False)
    # expert-0 w1, group-major contiguous pack:
    # w1z[g, p, j, dc*128+c] = w1[0][dc*128+p, (g*FG+j)*128+c]
    w1z = nc.declare_dram_parameter(
        "w1z", [FC // 4, P, 4, DC * P], BF16, isOutput=False)
    # packed: w1s[k, p, dc, f] = w1[k, dc*128 + p, f]   (used for k >= 1)
    w1s = nc.declare_dram_parameter("w1s", [K, P, DC, DF], BF16, isOutput=False)
    # packed: w2s[k, p, fc, d] = w2[k, fc*128 + p, d]
    w2s = nc.declare_dram_parameter("w2s", [K, P, FC, D], BF16, isOutput=False)
    # gwp[p, tt*K + k] = softmax gate weight for token tt*128+p, expert k
    gwp = nc.declare_dram_parameter("gwp", [P, TT * K], F32, isOutput=False)
    # b1pk[p, k*FC + fc] = b1[k, fc*128 + p]
    b1pk = nc.declare_dram_parameter("b1pk", [P, K * FC], F32, isOutput=False)
    out = nc.declare_dram_parameter("out", [T, D], BF16, isOutput=True)

    mult = mybir.AluOpType.mult
    add = mybir.AluOpType.add
    gelu_fn = mybir.ActivationFunctionType.Gelu_apprx_tanh

    with tile.TileContext(nc) as tc:
        with ExitStack() as ctx:
            persist = ctx.enter_context(tc.tile_pool(name="persist", bufs=1))
            w1gp = ctx.enter_context(tc.tile_pool(name="w1gp", bufs=NG))
            w1p = ctx.enter_context(tc.tile_pool(name="w1p", bufs=2))
            w2p = ctx.enter_context(tc.tile_pool(name="w2p", bufs=2))
            hp = ctx.enter_context(tc.tile_pool(name="hp", bufs=FC))
            ob = ctx.enter_context(tc.tile_pool(name="ob", bufs=4))
            psA = ctx.enter_context(tc.tile_pool(name="psA", bufs=4, space="PSUM"))
            psB = ctx.enter_context(tc.tile_pool(name="psB", bufs=3, space="PSUM"))

            # ---- critical-path loads ----
            # All SDMA rings share HBM bandwidth concurrently at packet
            # granularity, and the list scheduler issues dependency-free
            # DMAs as early as possible -- so the first matmul's critical
            # set gets full bandwidth only if every other transfer is held
            # back by a REAL dependency.  xt1-3 serialize behind the xt0
            # halves via sync-ring FIFO; w1g1-3/gw/w2/w1 prefetches are
            # gated by 2-element "gadget" copies (see _head_feed) that give
            # each DMA a WAW dependency on an early compute result.
            # (tile_wait_until stamps were tried and poisoned the whole
            # schedule: +43ns on every matmul; half-size first tiles were
            # tried and lost to downstream x-stream stalls.)
            w1g = [
                w1gp.tile([P, FG, DC * P], BF16, tag="w1g", name=f"w1g_{g}")
                for g in range(NG)
            ]
            xt = [
                persist.tile([P, DC, 512], BF16, tag=f"xt{tcc}", name=f"xt{tcc}")
                for tcc in range(TC)
            ]
            b1_sb = persist.tile([P, K * FC], F32, tag="b1", name="b1_sb")
            gw_sb = persist.tile([P, TT * K], F32, tag="gw", name="gw_sb")
            # w1g0 rides the scalar HWDGE ring (~0.6us start latency) --
            # on the gpsimd SWDGE path its ~2us fixed start cost gated the
            # first matmul.
            nc.scalar.dma_start(w1g[0][:], w1z[0])
            nc.sync.dma_start(xt[0][:], xq[0])
            nc.gpsimd.dma_start(b1_sb[:], b1pk[:])
            for tcc in range(1, TC):
                nc.sync.dma_start(xt[tcc][:], xq[tcc])

            acc = [
                persist.tile([P, D], F32, tag=f"acc{t}", name=f"acc{t}")
                for t in range(TT)
            ]

            # ---- PE + ACT warmup during the DMA head ----
            # The PE runs ~2x slower until ~3.4us of sustained activity
            # (HAM clock gate); dummy matmuls on a zeroed scratch tile keep
            # it busy while the critical DMAs land.  A dummy gelu
            # pre-triggers the ~1.3us ACT_TABLE_LOAD for the gelu set.
            # Full-array warmup tiles: [16,16] dummy matmuls left the HAM
            # clock-gate cold (first ~8 real matmuls measured at the 1.2GHz
            # cold-latency formula); 128x128 tiles register as real
            # activity.
            warm = persist.tile([P, P], BF16, tag="warm", name="warm")
            nc.vector.memset(warm[:], 0)
            warm_ps = psB.tile([P, 512], F32, tag="po", name="warm_ps")
            for r in range(30):
                nc.tensor.matmul(
                    warm_ps[:, 0:P], warm[:], warm[:],
                    start=True, stop=True,
                )
            warm_h = persist.tile([P, 16], BF16, tag="warmh", name="warm_h")
            nc.scalar.activation(warm_h[:], warm[:, 0:16], gelu_fn)

            # x chunks: (tile, src column offset, h column offset, width).
            CHUNKS = [(xt[tcc], 0, tcc * 512, 512) for tcc in range(TC)]
            NCH = len(CHUNKS)

            def emit_l1(k, stat, after_group=None, order=None):
                """h[F,T] = gelu(W1.T @ x + b1); stat(dc, fc) -> stationary AP.

                dc innermost: each psum group completes in DC consecutive
                matmuls and its gelu issues immediately -- smooth ACT cadence.
                `order` (expert 0) sequences groups so each head DMA's
                deadline falls as late as possible.
                """
                ht = [
                    hp.tile([P, T], BF16, tag="h", name=f"h_{k}_{fc}")
                    for fc in range(FC)
                ]
                if order is None:
                    order = [
                        (fc, ch) for fc in range(FC) for ch in range(NCH)
                    ]
                for gidx, (fc, ch) in enumerate(order):
                    xtile, soff, hoff, width = CHUNKS[ch]
                    ph = psA.tile(
                        [P, 512], F32, tag="ph", name=f"ph_{k}_{fc}_{ch}"
                    )
                    for dc in range(DC):
                        nc.tensor.matmul(
                            ph[:, 0:width],
                            stat(dc, fc),
                            xtile[:, dc, soff:soff + width],
                            start=(dc == 0),
                            stop=(dc == DC - 1),
                        )
                    nc.scalar.activation(
                        ht[fc][:, hoff:hoff + width], ph[:, 0:width], gelu_fn,
                        bias=b1_sb[:, k * FC + fc:k * FC + fc + 1],
                    )
                    if after_group is not None:
                        after_group(gidx + 1, ph, width)
                return ht

            def emit_l2(k, ht, w2t):
                """eo[T,D] = h.T @ W2 ; acc (+)= eo * gw[:,k]; store when k==K-1."""
                for tt in range(TT):
                    po = psB.tile([P, 512], F32, tag="po", name=f"po_{k}_{tt}")
                    for fc in range(FC):
                        nc.tensor.matmul(
                            po[:, 0:D],
                            ht[fc][:, tt * P:(tt + 1) * P],
                            w2t[:, fc, :],
                            start=(fc == 0),
                            stop=(fc == FC - 1),
                        )
                    g = gw_sb[:, tt * K + k:tt * K + k + 1]
                    if k == K - 1 and K == 1:
                        o = ob.tile([P, D], BF16, tag="o", name=f"o_{tt}")
                        nc.vector.tensor_scalar_mul(o[:], po[:, 0:D], g)
                        eng = nc.sync if tt % 2 == 0 else nc.scalar
                        eng.dma_start(out[tt * P:(tt + 1) * P, :], o[:])
                    elif k == 0:
                        nc.vector.tensor_scalar_mul(acc[tt][:], po[:, 0:D], g)
                    elif k < K - 1:
                        nc.vector.scalar_tensor_tensor(
                            acc[tt][:], po[:, 0:D], g,
                            acc[tt][:], op0=mult, op1=add,
                        )
                    else:
                        o = ob.tile([P, D], BF16, tag="o", name=f"o_{tt}")
                        nc.vector.scalar_tensor_tensor(
                            o[:], po[:, 0:D], g,
                            acc[tt][:], op0=mult, op1=add,
                        )
                        eng = nc.sync if tt % 2 == 0 else nc.scalar
                        eng.dma_start(out[tt * P:(tt + 1) * P, :], o[:])

            # ---- weight tiles + prefetch scheduling ----
            # Queue-program placement does NOT throttle a DMA issue -- the
            # list scheduler hoists dependency-free DMAs to the queue head
            # (v4/v6/v8 lesson: early 2MB prefetches stole HBM bandwidth
            # from the critical head loads).  Gadget-copy WAW deps are the
            # reliable throttle.
            w2t = [
                w2p.tile([P, FC, D], BF16, tag="w2", name=f"w2_{k}")
                for k in range(K)
            ]
            w1t = [None] + [
                w1p.tile([P, DC, DF], BF16, tag="w1", name=f"w1_{k}")
                for k in range(1, K)
            ]

            def _head_feed(gidx, ph, width):
                # Gate the remaining head transfers on early psum groups:
                # a 2-element vector copy reads the group's psum and writes
                # a corner of the DMA destination, so the DMA's WAW dep
                # delays its transfer until the critical set is in use.
                if 2 <= gidx <= NG and gidx - 1 < NG:
                    g = gidx - 1
                    nc.vector.tensor_scalar_add(
                        w1g[g][0:1, 0, 0:2], ph[0:1, 0:2], 0.0)
                    nc.gpsimd.dma_start(w1g[g][:], w1z[g])
                elif gidx == NG + 1:
                    nc.vector.tensor_scalar_add(
                        gw_sb[0:1, 0:2], ph[0:1, 0:2], 0.0)
                    nc.gpsimd.dma_start(gw_sb[:], gwp[:])

            def _stat0(dc, fc):
                return w1g[fc // FG][:, fc % FG, dc * P:(dc + 1) * P]

            # fc-quadrant-major order: the first FG groups need only
            # w1g0 + xt0; each later input tile's deadline is a full
            # quadrant (~13.6us) out.
            order0 = [
                (fc, ch)
                for g in range(NG)
                for ch in range(NCH)
                for fc in range(g * FG, (g + 1) * FG)
            ]
            ht = emit_l1(0, _stat0, after_group=_head_feed, order=order0)

            # Next-expert weight streams on the sync ring (FIFO behind the
            # xt tiles), each gated by a gadget copy off an h tile whose
            # last write lands progressively deeper into expert-0 L1.
            nc.vector.tensor_scalar_add(
                w2t[0][0:1, 0, 0:2], ht[1][0:1, 0:2], 0.0)
            nc.sync.dma_start(w2t[0][:], w2s[0])
            if K > 1:
                nc.vector.tensor_scalar_add(
                    w1t[1][0:1, 0, 0:2], ht[5][0:1, 0:2], 0.0)
                nc.sync.dma_start(w1t[1][:], w1s[1])
                nc.vector.tensor_scalar_add(
                    w2t[1][0:1, 0, 0:2], ht[9][0:1, 0:2], 0.0)
                nc.sync.dma_start(w2t[1][:], w2s[1])
            if K > 2:
                nc.vector.tensor_scalar_add(
                    w1t[2][0:1, 0, 0:2], ht[13][0:1, 0:2], 0.0)
                nc.sync.dma_start(w1t[2][:], w1s[2])
            emit_l2(0, ht, w2t[0])

            for k in range(1, K):
                if k >= 2:
                    nc.gpsimd.dma_start(w2t[k][:], w2s[k])
                if k + 1 < K and k + 1 >= 3:
                    nc.gpsimd.dma_start(w1t[k + 1][:], w1s[k + 1])
                ht = emit_l1(
                    k,
                    lambda dc, fc, w=w1t[k]: w[:, dc, fc * P:(fc + 1) * P],
                )
                emit_l2(k, ht, w2t[k])

    _split_multi_waits(nc)
    return nc


# ---------------------------------------------------------------------------
# Host wrapper
# ---------------------------------------------------------------------------
_NC_CACHE: dict = {}


def _get_nc(K: int, T: int, D: int, DF: int):
    key = (K, T, D, DF)
    if key not in _NC_CACHE:
        _NC_CACHE[key] = build_moe_kernel(K, T, D, DF)
    return _NC_CACHE[key]


def _softmax(x, axis=-1):
    m = np.max(x, axis=axis, keepdims=True)
    e = np.exp(x - m)
    return e / np.sum(e, axis=axis, keepdims=True)


def run(inputs: dict, trace: bool = False, tmpdir: str | None = None):
    x = np.asarray(inputs["x"], dtype=np.float32)
    gate_w = np.asarray(inputs["gate_w"], dtype=np.float32)
    gate_b = np.asarray(inputs["gate_b"], dtype=np.float32)
    w1 = np.asarray(inputs["w1"], dtype=np.float32)
    b1 = np.asarray(inputs["b1"], dtype=np.float32)
    w2 = np.asarray(inputs["w2"], dtype=np.float32)
    b2 = np.asarray(inputs["b2"], dtype=np.float32)
    K = int(inputs["num_available"])

    B, S, D = x.shape
    DF = w1.shape[2]
    Ttot = B * S
    T = Ttot // N_CORES
    DC = D // P
    FC = DF // P
    TT = T // P
    TC = T // 512

    # Coarse routing on host (tiny): gate applied to the global token sum.
    ksum = x.sum(axis=(0, 1))
    coarse = gate_w @ ksum + gate_b
    idx = np.argsort(-coarse, kind="stable")[:K]

    gws = gate_w[idx]                      # [K, D]
    gbs = gate_b[idx]                      # [K]
    b1s = np.ascontiguousarray(b1[idx], dtype=np.float32)              # [K,DF]
    b2s = np.ascontiguousarray(b2[idx], dtype=np.float32)              # [K,D]

    # per-token softmax gating on host (0.02% of the FLOPs)
    xf = x.reshape(Ttot, D)
    logits = xf @ gws.T + gbs[None, :]                                 # [Ttot,K]
    gw = _softmax(logits, axis=1).astype(np.float32)

    # packed weights
    w1sel = np.ascontiguousarray(
        w1[idx].reshape(K, DC, P, DF).transpose(0, 2, 1, 3)
    ).astype(ml_dtypes.bfloat16)                                       # [K,P,DC,DF]
    w2sel = np.ascontiguousarray(
        w2[idx].reshape(K, FC, P, D).transpose(0, 2, 1, 3)
    ).astype(ml_dtypes.bfloat16)                                       # [K,P,FC,D]
    # expert-0 w1, group-major contiguous pack: [NG, P, FG, DC*128]
    w1z = np.ascontiguousarray(
        w1[idx[0]].reshape(DC, P, FC // 4, 4, P).transpose(2, 1, 3, 0, 4)
        .reshape(FC // 4, P, 4, DC * P)
    ).astype(ml_dtypes.bfloat16)
    b1pk = np.ascontiguousarray(
        b1s.reshape(K, FC, P).transpose(2, 0, 1).reshape(P, K * FC), dtype=np.float32
    )

    nc = _get_nc(K, T, D, DF)
    in_maps = []
    for c in range(N_CORES):
        gwc = gw[c * T:(c + 1) * T]  # [T, K]
        gwp = np.ascontiguousarray(
            gwc.reshape(TT, P, K).transpose(1, 0, 2).reshape(P, TT * K),
            dtype=np.float32,
        )
        # x packed chunk-major [TC, P, DC, 512]
        xc = xf[c * T:(c + 1) * T]
        xqc = np.ascontiguousarray(
            xc.reshape(TC, 512, DC, P).transpose(0, 3, 2, 1)
        ).astype(ml_dtypes.bfloat16)
        in_maps.append({
            "xq": xqc,
            "w1z": w1z,
            "w1s": w1sel,
            "w2s": w2sel,
            "gwp": gwp,
            "b1pk": b1pk,
        })

    res = run_bass_kernel_spmd(
        nc, in_maps, list(range(N_CORES)), trace=trace, tmpdir=tmpdir
    )
    outp = np.concatenate(
        [np.asarray(res.results[c]["out"]) for c in range(N_CORES)], axis=0
    ).astype(np.float32).reshape(B, S, D)

    # b2 contribution (zero in this problem's inputs; exact host-side fallback)
    if np.any(b2s):
        outp = outp + (gw @ b2s).reshape(B, S, D)

    return outp, res


def kernel(**inputs) -> np.ndarray:
    outp, _ = run(inputs, trace=False)
    return outp


# revision 31
# speedup vs baseline: 1.0086x; 1.0086x over previous
"""Mixture-of-Experts Trainium2 kernel (8-core SPMD, token-sharded, bf16).

v6: head restructure guided by the NTFF traces of v4 (470.2us) and the
failed v5 (478.9us):
  * v4's PE only went dense at ~25us: the gpsimd queue ran ahead and
    issued 8MB of next-expert prefetches whose SDMA packets round-robin-
    stole HBM bandwidth from the critical xt loads.
  * v5 split w1_0 into 16 small SWDGE DMAs -- issue-rate limited (~0.9us
    per issue, stretching to 3us+ under descriptor-ring backlog), which
    starved L1 even harder.
  * v6: 4 xt tiles [P,DC,512] and 4 w1_0 fc-group tiles [P,4,DC*128],
    each host-packed so a tile is 4KB-contiguous per partition (128 fat
    descriptors per DMA).  Critical tiles go first on each ring (sync:
    xt0,xt2; scalar: xt1,xt3; gpsimd: w1g0-3); big prefetches are emitted
    behind them so ring FIFO keeps the critical window clean.  gw/b1 ride
    the otherwise-idle vector queue.  Warmup extended to 96 matmuls to
    keep the HAM clock-gate warm until the first real matmul (~10.5us).
  * tail kept from v4 (measured at its floor: ~0.75us combine + 0.6us
    issue + ~3us DMA completion receipt); output DMAs alternate rings.
"""

import numpy as np
import ml_dtypes
from contextlib import ExitStack

import bass_rust as _bass_rust
import concourse.bass as bass
import concourse.mybir as mybir
import concourse.tile as tile
from concourse.bass_utils import run_bass_kernel_spmd

BF16 = mybir.dt.bfloat16
F32 = mybir.dt.float32
N_CORES = 8
P = 128


# ---------------------------------------------------------------------------
# Workaround for walrus "Too many sync wait commands" (see baseline).
# ---------------------------------------------------------------------------
_split_ctr = [0]


def _split_multi_waits(nc):
    for f in nc.m.functions:
        for blk in f.blocks:
            insts = blk.instructions
            i = 0
            while i < len(insts):
                inst = insts[i]
                si = getattr(inst, "sync_info", None)
                waits = list(si.on_wait) if si is not None and si.on_wait else []
                if len(waits) > 1:
                    si.on_wait = waits[-1:]
                    for w in waits[:-1]:
                        _split_ctr[0] += 1
                        ev = mybir.InstEventSemaphore(
                            name=f"I-wsplit-{_split_ctr[0]}", ins=[], outs=[]
                        )
                        ev.engine = inst.engine
                        ev.sync_info = _bass_rust.SyncInfo(
                            on_wait=[w], on_update=[]
                        )
                        insts.insert(i, ev)
                        i += 1
                i += 1


# ---------------------------------------------------------------------------
# Device kernel
# ---------------------------------------------------------------------------
def build_moe_kernel(K: int, T: int, D: int, DF: int):
    assert T % 512 == 0 and D % P == 0 and DF % P == 0
    TT = T // P       # 128-token tiles
    TC = T // 512     # 512-token chunks
    DC = D // P       # D chunks of 128
    FC = DF // P      # F chunks of 128
    FG = 4            # fc per w1_0 head tile
    NG = FC // FG

    nc = bass.Bass("TRN2", target_bir_lowering=False)

    # x packed chunk-major so each xt tile is one CONTIGUOUS 512KB block
    # (4KB reads strided 16KB measured only ~107GB/s -- 25% HBM row use):
    # xq[tcc, p, dc, j] = x[tcc*512+j, dc*128+p]
    xq = nc.declare_dram_parameter("xq", [TC, P, DC, 512], BF16, isOutput=False)
    # expert-0 w1, group-major contiguous pack:
    # w1z[g, p, j, dc*128+c] = w1[0][dc*128+p, (g*FG+j)*128+c]
    w1z = nc.declare_dram_parameter(
        "w1z", [FC // 4, P, 4, DC * P], BF16, isOutput=False)
    # packed: w1s[k, p, dc, f] = w1[k, dc*128 + p, f]   (used for k >= 1)
    w1s = nc.declare_dram_parameter("w1s", [K, P, DC, DF], BF16, isOutput=False)
    # packed: w2s[k, p, fc, d] = w2[k, fc*128 + p, d]
    w2s = nc.declare_dram_parameter("w2s", [K, P, FC, D], BF16, isOutput=False)
    # gwp[p, tt*K + k] = softmax gate weight for token tt*128+p, expert k
    gwp = nc.declare_dram_parameter("gwp", [P, TT * K], F32, isOutput=False)
    # b1pk[p, k*FC + fc] = b1[k, fc*128 + p]
    b1pk = nc.declare_dram_parameter("b1pk", [P, K * FC], F32, isOutput=False)
    out = nc.declare_dram_parameter("out", [T, D], BF16, isOutput=True)

    mult = mybir.AluOpType.mult
    add = mybir.AluOpType.add
    gelu_fn = mybir.ActivationFunctionType.Gelu_apprx_tanh

    with tile.TileContext(nc) as tc:
        with ExitStack() as ctx:
            persist = ctx.enter_context(tc.tile_pool(name="persist", bufs=1))
            w1gp = ctx.enter_context(tc.tile_pool(name="w1gp", bufs=NG))
            w1p = ctx.enter_context(tc.tile_pool(name="w1p", bufs=2))
            w2p = ctx.enter_context(tc.tile_pool(name="w2p", bufs=2))
            hp = ctx.enter_context(tc.tile_pool(name="hp", bufs=FC))
            ob = ctx.enter_context(tc.tile_pool(name="ob", bufs=4))
            psA = ctx.enter_context(tc.tile_pool(name="psA", bufs=4, space="PSUM"))
            psB = ctx.enter_context(tc.tile_pool(name="psB", bufs=3, space="PSUM"))

            # ---- critical-path loads ----
            # All SDMA rings share HBM bandwidth concurrently at packet
            # granularity, and the list scheduler issues dependency-free
            # DMAs as early as possible -- so the first matmul's critical
            # set gets full bandwidth only if every other transfer is held
            # back by a REAL dependency.  xt1-3 serialize behind the xt0
            # halves via sync-ring FIFO; w1g1-3/gw/w2/w1 prefetches are
            # gated by 2-element "gadget" copies (see _head_feed) that give
            # each DMA a WAW dependency on an early compute result.
            # (tile_wait_until stamps were tried and poisoned the whole
            # schedule: +43ns on every matmul; half-size first tiles were
            # tried and lost to downstream x-stream stalls.)
            w1g = [
                w1gp.tile([P, FG, DC * P], BF16, tag="w1g", name=f"w1g_{g}")
                for g in range(NG)
            ]
            xt = [
                persist.tile([P, DC, 512], BF16, tag=f"xt{tcc}", name=f"xt{tcc}")
                for tcc in range(TC)
            ]
            b1_sb = persist.tile([P, K * FC], F32, tag="b1", name="b1_sb")
            gw_sb = persist.tile([P, TT * K], F32, tag="gw", name="gw_sb")
            # w1g0 rides the scalar HWDGE ring (~0.6us start latency) --
            # on the gpsimd SWDGE path its ~2us fixed start cost gated the
            # first matmul.
            nc.scalar.dma_start(w1g[0][:], w1z[0])
            nc.sync.dma_start(xt[0][:], xq[0])
            nc.gpsimd.dma_start(b1_sb[:], b1pk[:])
            for tcc in range(1, TC):
                nc.sync.dma_start(xt[tcc][:], xq[tcc])

            acc = [
                persist.tile([P, D], F32, tag=f"acc{t}", name=f"acc{t}")
                for t in range(TT)
            ]

            # ---- PE + ACT warmup during the DMA head ----
            # The PE runs ~2x slower until ~3.4us of sustained activity
            # (HAM clock gate); dummy matmuls on a zeroed scratch tile keep
            # it busy while the critical DMAs land.  A dummy gelu
            # pre-triggers the ~1.3us ACT_TABLE_LOAD for the gelu set.
            # Full-array warmup tiles: [16,16] dummy matmuls left the HAM
            # clock-gate cold (first ~8 real matmuls measured at the 1.2GHz
            # cold-latency formula); 128x128 tiles register as real
            # activity.
            warm = persist.tile([P, P], BF16, tag="warm", name="warm")
            nc.vector.memset(warm[:], 0)
            warm_ps = psB.tile([P, 512], F32, tag="po", name="warm_ps")
            for r in range(50):
                nc.tensor.matmul(
                    warm_ps[:, 0:P], warm[:], warm[:],
                    start=True, stop=True,
                )
            warm_h = persist.tile([P, 16], BF16, tag="warmh", name="warm_h")
            nc.scalar.activation(warm_h[:], warm[:, 0:16], gelu_fn)

            # x chunks: (tile, src column offset, h column offset, width).
            CHUNKS = [(xt[tcc], 0, tcc * 512, 512) for tcc in range(TC)]
            NCH = len(CHUNKS)

            def emit_l1(k, stat, after_group=None, order=None):
                """h[F,T] = gelu(W1.T @ x + b1); stat(dc, fc) -> stationary AP.

                dc innermost: each psum group completes in DC consecutive
                matmuls and its gelu issues immediately -- smooth ACT cadence.
                `order` (expert 0) sequences groups so each head DMA's
                deadline falls as late as possible.
                """
                ht = [
                    hp.tile([P, T], BF16, tag="h", name=f"h_{k}_{fc}")
                    for fc in range(FC)
                ]
                if order is None:
                    order = [
                        (fc, ch) for fc in range(FC) for ch in range(NCH)
                    ]
                for gidx, (fc, ch) in enumerate(order):
                    xtile, soff, hoff, width = CHUNKS[ch]
                    ph = psA.tile(
                        [P, 512], F32, tag="ph", name=f"ph_{k}_{fc}_{ch}"
                    )
                    for dc in range(DC):
                        nc.tensor.matmul(
                            ph[:, 0:width],
                            stat(dc, fc),
                            xtile[:, dc, soff:soff + width],
                            start=(dc == 0),
                            stop=(dc == DC - 1),
                        )
                    nc.scalar.activation(
                        ht[fc][:, hoff:hoff + width], ph[:, 0:width], gelu_fn,
                        bias=b1_sb[:, k * FC + fc:k * FC + fc + 1],
                    )
                    if after_group is not None:
                        after_group(gidx + 1, ph, width)
                return ht

            def emit_l2(k, ht, w2t):
                """eo[T,D] = h.T @ W2 ; acc (+)= eo * gw[:,k]; store when k==K-1."""
                for tt in range(TT):
                    po = psB.tile([P, 512], F32, tag="po", name=f"po_{k}_{tt}")
                    for fc in range(FC):
                        nc.tensor.matmul(
                            po[:, 0:D],
                            ht[fc][:, tt * P:(tt + 1) * P],
                            w2t[:, fc, :],
                            start=(fc == 0),
                            stop=(fc == FC - 1),
                        )
                    g = gw_sb[:, tt * K + k:tt * K + k + 1]
                    if k == K - 1 and K == 1:
                        o = ob.tile([P, D], BF16, tag="o", name=f"o_{tt}")
                        nc.vector.tensor_scalar_mul(o[:], po[:, 0:D], g)
                        eng = nc.sync if tt % 2 == 0 else nc.scalar
                        eng.dma_start(out[tt * P:(tt + 1) * P, :], o[:])
                    elif k == 0:
                        nc.vector.tensor_scalar_mul(acc[tt][:], po[:, 0:D], g)
                    elif k < K - 1:
                        nc.vector.scalar_tensor_tensor(
                            acc[tt][:], po[:, 0:D], g,
                            acc[tt][:], op0=mult, op1=add,
                        )
                    else:
                        o = ob.tile([P, D], BF16, tag="o", name=f"o_{tt}")
                        nc.vector.scalar_tensor_tensor(
                            o[:], po[:, 0:D], g,
                            acc[tt][:], op0=mult, op1=add,
                        )
                        eng = nc.sync if tt % 2 == 0 else nc.scalar
                        eng.dma_start(out[tt * P:(tt + 1) * P, :], o[:])

            # ---- weight tiles + prefetch scheduling ----
            # Queue-program placement does NOT throttle a DMA issue -- the
            # list scheduler hoists dependency-free DMAs to the queue head
            # (v4/v6/v8 lesson: early 2MB prefetches stole HBM bandwidth
            # from the critical head loads).  Gadget-copy WAW deps are the
            # reliable throttle.
            w2t = [
                w2p.tile([P, FC, D], BF16, tag="w2", name=f"w2_{k}")
                for k in range(K)
            ]
            w1t = [None] + [
                w1p.tile([P, DC, DF], BF16, tag="w1", name=f"w1_{k}")
                for k in range(1, K)
            ]

            def _head_feed(gidx, ph, width):
                # Gate the remaining head transfers on early psum groups:
                # a 2-element vector copy reads the group's psum and writes
                # a corner of the DMA destination, so the DMA's WAW dep
                # delays its transfer until the critical set is in use.
                if 2 <= gidx <= NG and gidx - 1 < NG:
                    g = gidx - 1
                    nc.vector.tensor_scalar_add(
                        w1g[g][0:1, 0, 0:2], ph[0:1, 0:2], 0.0)
                    nc.gpsimd.dma_start(w1g[g][:], w1z[g])
                elif gidx == NG + 1:
                    nc.vector.tensor_scalar_add(
                        gw_sb[0:1, 0:2], ph[0:1, 0:2], 0.0)
                    nc.gpsimd.dma_start(gw_sb[:], gwp[:])

            def _stat0(dc, fc):
                return w1g[fc // FG][:, fc % FG, dc * P:(dc + 1) * P]

            # fc-quadrant-major order: the first FG groups need only
            # w1g0 + xt0; each later input tile's deadline is a full
            # quadrant (~13.6us) out.
            order0 = [
                (fc, ch)
                for g in range(NG)
                for ch in range(NCH)
                for fc in range(g * FG, (g + 1) * FG)
            ]
            ht = emit_l1(0, _stat0, after_group=_head_feed, order=order0)

            # Next-expert weight streams on the sync ring (FIFO behind the
            # xt tiles), each gated by a gadget copy off an h tile whose
            # last write lands progressively deeper into expert-0 L1.
            nc.vector.tensor_scalar_add(
                w2t[0][0:1, 0, 0:2], ht[1][0:1, 0:2], 0.0)
            nc.sync.dma_start(w2t[0][:], w2s[0])
            if K > 1:
                nc.vector.tensor_scalar_add(
                    w1t[1][0:1, 0, 0:2], ht[5][0:1, 0:2], 0.0)
                nc.sync.dma_start(w1t[1][:], w1s[1])
                nc.vector.tensor_scalar_add(
                    w2t[1][0:1, 0, 0:2], ht[9][0:1, 0:2], 0.0)
                nc.sync.dma_start(w2t[1][:], w2s[1])
            if K > 2:
                nc.vector.tensor_scalar_add(
                    w1t[2][0:1, 0, 0:2], ht[13][0:1, 0:2], 0.0)
                nc.sync.dma_start(w1t[2][:], w1s[2])
            emit_l2(0, ht, w2t[0])

            for k in range(1, K):
                if k >= 2:
                    nc.gpsimd.dma_start(w2t[k][:], w2s[k])
                if k + 1 < K and k + 1 >= 3:
                    nc.gpsimd.dma_start(w1t[k + 1][:], w1s[k + 1])
                ht = emit_l1(
                    k,
                    lambda dc, fc, w=w1t[k]: w[:, dc, fc * P:(fc + 1) * P],
                )
                emit_l2(k, ht, w2t[k])

    _split_multi_waits(nc)
    return nc


# ---------------------------------------------------------------------------
# Host wrapper
# ---------------------------------------------------------------------------
_NC_CACHE: dict = {}


def _get_nc(K: int, T: int, D: int, DF: int):
    key = (K, T, D, DF)
    if key not in _NC_CACHE:
        _NC_CACHE[key] = build_moe_kernel(K, T, D, DF)
    return _NC_CACHE[key]


def _softmax(x, axis=-1):
    m = np.max(x, axis=axis, keepdims=True)
    e = np.exp(x - m)
    return e / np.sum(e, axis=axis, keepdims=True)


def run(inputs: dict, trace: bool = False, tmpdir: str | None = None):
    x = np.asarray(inputs["x"], dtype=np.float32)
    gate_w = np.asarray(inputs["gate_w"], dtype=np.float32)
    gate_b = np.asarray(inputs["gate_b"], dtype=np.float32)
    w1 = np.asarray(inputs["w1"], dtype=np.float32)
    b1 = np.asarray(inputs["b1"], dtype=np.float32)
    w2 = np.asarray(inputs["w2"], dtype=np.float32)
    b2 = np.asarray(inputs["b2"], dtype=np.float32)
    K = int(inputs["num_available"])

    B, S, D = x.shape
    DF = w1.shape[2]
    Ttot = B * S
    T = Ttot // N_CORES
    DC = D // P
    FC = DF // P
    TT = T // P
    TC = T // 512

    # Coarse routing on host (tiny): gate applied to the global token sum.
    ksum = x.sum(axis=(0, 1))
    coarse = gate_w @ ksum + gate_b
    idx = np.argsort(-coarse, kind="stable")[:K]

    gws = gate_w[idx]                      # [K, D]
    gbs = gate_b[idx]                      # [K]
    b1s = np.ascontiguousarray(b1[idx], dtype=np.float32)              # [K,DF]
    b2s = np.ascontiguousarray(b2[idx], dtype=np.float32)              # [K,D]

    # per-token softmax gating on host (0.02% of the FLOPs)
    xf = x.reshape(Ttot, D)
    logits = xf @ gws.T + gbs[None, :]                                 # [Ttot,K]
    gw = _softmax(logits, axis=1).astype(np.float32)

    # packed weights
    w1sel = np.ascontiguousarray(
        w1[idx].reshape(K, DC, P, DF).transpose(0, 2, 1, 3)
    ).astype(ml_dtypes.bfloat16)                                       # [K,P,DC,DF]
    w2sel = np.ascontiguousarray(
        w2[idx].reshape(K, FC, P, D).transpose(0, 2, 1, 3)
    ).astype(ml_dtypes.bfloat16)                                       # [K,P,FC,D]
    # expert-0 w1, group-major contiguous pack: [NG, P, FG, DC*128]
    w1z = np.ascontiguousarray(
        w1[idx[0]].reshape(DC, P, FC // 4, 4, P).transpose(2, 1, 3, 0, 4)
        .reshape(FC // 4, P, 4, DC * P)
    ).astype(ml_dtypes.bfloat16)
    b1pk = np.ascontiguousarray(
        b1s.reshape(K, FC, P).transpose(2, 0, 1).reshape(P, K * FC), dtype=np.float32
    )

    nc = _get_nc(K, T, D, DF)
    in_maps = []
    for c in range(N_CORES):
        gwc = gw[c * T:(c + 1) * T]  # [T, K]
        gwp = np.ascontiguousarray(
            gwc.reshape(TT, P, K).transpose(1, 0, 2).reshape(P, TT * K),
            dtype=np.float32,
        )
        # x packed chunk-major [TC, P, DC, 512]
        xc = xf[c * T:(c + 1) * T]
        xqc = np.ascontiguousarray(
            xc.reshape(TC, 512, DC, P).transpose(0, 3, 2, 1)
        ).astype(ml_dtypes.bfloat16)
        in_maps.append({
            "xq": xqc,
            "w1z": w1z,
            "w1s": w1sel,
            "w2s": w2sel,
            "gwp": gwp,
            "b1pk": b1pk,
        })

    res = run_bass_kernel_spmd(
        nc, in_maps, list(range(N_CORES)), trace=trace, tmpdir=tmpdir
    )
    outp = np.concatenate(
        [np.asarray(res.results[c]["out"]) for c in range(N_CORES)], axis=0
    ).astype(np.float32).reshape(B, S, D)

    # b2 contribution (zero in this problem's inputs; exact host-side fallback)
    if np.any(b2s):
        outp = outp + (gw @ b2s).reshape(B, S, D)

    return outp, res


def kernel(**inputs) -> np.ndarray:
    outp, _ = run(inputs, trace=False)
    return outp


# revision 34
# speedup vs baseline: 1.0091x; 1.0005x over previous
"""Mixture-of-Experts Trainium2 kernel (8-core SPMD, token-sharded, bf16).

v17 (from the 469.6us v4 baseline; fast-state ~460.3us):
  * Dense matmul region runs at the warm PE roofline (2048 N=512 bf16
    matmuls x ~216ns) with zero mid-kernel PE gaps.
  * Head: the first matmul's 1MB critical set (xt0 on the sync HWDGE
    ring, w1_0's first fc-quad on the scalar HWDGE ring) transfers at
    the measured ~107GB/s-per-DMA cap with nothing competing: xt1-3
    serialize behind xt0 via sync-ring FIFO, and every other transfer
    (w1g1-3, gw, next-expert w1/w2 streams) is gated by a 2-element
    "gadget" copy that gives its DMA a WAW dependency on an early psum
    group / h tile.  Plain queue-program placement does NOT delay a
    dependency-free DMA (the list scheduler hoists it), and
    tile_wait_until stamps poison the whole schedule (+43ns per matmul).
    Expert-0 L1 iterates fc-quadrant-major so each head tile's deadline
    is a full quadrant (~13.6us) out.
  * Warmup: 50 full-array [128x128] dummy matmuls keep the HAM clock
    gate warm until the first real matmul (~13us).  [16x16] warm tiles
    do NOT register as activity -- real matmuls then pay the 1.2GHz
    cold-clock formula for ~2us.
  * Tail is at its floor: combine (0.75us DVE) + DMA issue (0.6us) +
    transfer + ~3us completion receipt.
  * Routing + per-token softmax gating (0.02% of FLOPs) run on host;
    weights are host-packed so every device tile is contiguous per
    partition (128 fat descriptors per DMA).
"""

import numpy as np
import ml_dtypes
from contextlib import ExitStack

import bass_rust as _bass_rust
import concourse.bass as bass
import concourse.mybir as mybir
import concourse.tile as tile
from concourse.bass_utils import run_bass_kernel_spmd

BF16 = mybir.dt.bfloat16
F32 = mybir.dt.float32
N_CORES = 8
P = 128


# ---------------------------------------------------------------------------
# Workaround for walrus "Too many sync wait commands" (see baseline).
# ---------------------------------------------------------------------------
_split_ctr = [0]


def _split_multi_waits(nc):
    for f in nc.m.functions:
        for blk in f.blocks:
            insts = blk.instructions
            i = 0
            while i < len(insts):
                inst = insts[i]
                si = getattr(inst, "sync_info", None)
                waits = list(si.on_wait) if si is not None and si.on_wait else []
                if len(waits) > 1:
                    si.on_wait = waits[-1:]
                    for w in waits[:-1]:
                        _split_ctr[0] += 1
                        ev = mybir.InstEventSemaphore(
                            name=f"I-wsplit-{_split_ctr[0]}", ins=[], outs=[]
                        )
                        ev.engine = inst.engine
                        ev.sync_info = _bass_rust.SyncInfo(
                            on_wait=[w], on_update=[]
                        )
                        insts.insert(i, ev)
                        i += 1
                i += 1


# ---------------------------------------------------------------------------
# Device kernel
# ---------------------------------------------------------------------------
def build_moe_kernel(K: int, T: int, D: int, DF: int):
    assert T % 512 == 0 and D % P == 0 and DF % P == 0
    TT = T // P       # 128-token tiles
    TC = T // 512     # 512-token chunks
    DC = D // P       # D chunks of 128
    FC = DF // P      # F chunks of 128
    FG = 4            # fc per w1_0 head tile
    NG = FC // FG

    nc = bass.Bass("TRN2", target_bir_lowering=False)

    # x packed chunk-major so each xt tile is one CONTIGUOUS 512KB block
    # (4KB reads strided 16KB measured only ~107GB/s -- 25% HBM row use):
    # xq[tcc, p, dc, j] = x[tcc*512+j, dc*128+p]
    xq = nc.declare_dram_parameter("xq", [TC, P, DC, 512], BF16, isOutput=False)
    # expert-0 w1, group-major contiguous pack:
    # w1z[g, p, j, dc*128+c] = w1[0][dc*128+p, (g*FG+j)*128+c]
    w1z = nc.declare_dram_parameter(
        "w1z", [FC // 4, P, 4, DC * P], BF16, isOutput=False)
    # packed: w1s[k, p, dc, f] = w1[k, dc*128 + p, f]   (used for k >= 1)
    w1s = nc.declare_dram_parameter("w1s", [K, P, DC, DF], BF16, isOutput=False)
    # packed: w2s[k, p, fc, d] = w2[k, fc*128 + p, d]
    w2s = nc.declare_dram_parameter("w2s", [K, P, FC, D], BF16, isOutput=False)
    # gwp[p, tt*K + k] = softmax gate weight for token tt*128+p, expert k
    gwp = nc.declare_dram_parameter("gwp", [P, TT * K], F32, isOutput=False)
    # b1pk[p, k*FC + fc] = b1[k, fc*128 + p]
    b1pk = nc.declare_dram_parameter("b1pk", [P, K * FC], F32, isOutput=False)
    out = nc.declare_dram_parameter("out", [T, D], BF16, isOutput=True)

    mult = mybir.AluOpType.mult
    add = mybir.AluOpType.add
    gelu_fn = mybir.ActivationFunctionType.Gelu_apprx_tanh

    with tile.TileContext(nc) as tc:
        with ExitStack() as ctx:
            persist = ctx.enter_context(tc.tile_pool(name="persist", bufs=1))
            w1gp = ctx.enter_context(tc.tile_pool(name="w1gp", bufs=NG))
            w1p = ctx.enter_context(tc.tile_pool(name="w1p", bufs=2))
            w2p = ctx.enter_context(tc.tile_pool(name="w2p", bufs=2))
            hp = ctx.enter_context(tc.tile_pool(name="hp", bufs=FC))
            ob = ctx.enter_context(tc.tile_pool(name="ob", bufs=4))
            psA = ctx.enter_context(tc.tile_pool(name="psA", bufs=4, space="PSUM"))
            psB = ctx.enter_context(tc.tile_pool(name="psB", bufs=3, space="PSUM"))

            # ---- critical-path loads ----
            # All SDMA rings share HBM bandwidth concurrently at packet
            # granularity, and the list scheduler issues dependency-free
            # DMAs as early as possible -- so the first matmul's critical
            # set gets full bandwidth only if every other transfer is held
            # back by a REAL dependency.  xt1-3 serialize behind the xt0
            # halves via sync-ring FIFO; w1g1-3/gw/w2/w1 prefetches are
            # gated by 2-element "gadget" copies (see _head_feed) that give
            # each DMA a WAW dependency on an early compute result.
            # (tile_wait_until stamps were tried and poisoned the whole
            # schedule: +43ns on every matmul; half-size first tiles were
            # tried and lost to downstream x-stream stalls.)
            w1g = [
                w1gp.tile([P, FG, DC * P], BF16, tag="w1g", name=f"w1g_{g}")
                for g in range(NG)
            ]
            xt = [
                persist.tile([P, DC, 512], BF16, tag=f"xt{tcc}", name=f"xt{tcc}")
                for tcc in range(TC)
            ]
            b1_sb = persist.tile([P, K * FC], F32, tag="b1", name="b1_sb")
            gw_sb = persist.tile([P, TT * K], F32, tag="gw", name="gw_sb")
            # w1g0 rides the scalar HWDGE ring (~0.6us start latency) --
            # on the gpsimd SWDGE path its ~2us fixed start cost gated the
            # first matmul.
            nc.scalar.dma_start(w1g[0][:], w1z[0])
            nc.sync.dma_start(xt[0][:], xq[0])
            nc.gpsimd.dma_start(b1_sb[:], b1pk[:])
            for tcc in range(1, TC):
                nc.sync.dma_start(xt[tcc][:], xq[tcc])

            acc = [
                persist.tile([P, D], F32, tag=f"acc{t}", name=f"acc{t}")
                for t in range(TT)
            ]

            # ---- PE + ACT warmup during the DMA head ----
            # The PE runs ~2x slower until ~3.4us of sustained activity
            # (HAM clock gate); dummy matmuls on a zeroed scratch tile keep
            # it busy while the critical DMAs land.  A dummy gelu
            # pre-triggers the ~1.3us ACT_TABLE_LOAD for the gelu set.
            # Full-array warmup tiles: [16,16] dummy matmuls left the HAM
            # clock-gate cold (first ~8 real matmuls measured at the 1.2GHz
            # cold-latency formula); 128x128 tiles register as real
            # activity.
            warm = persist.tile([P, P], BF16, tag="warm", name="warm")
            nc.vector.memset(warm[:], 0)
            warm_ps = psB.tile([P, 512], F32, tag="po", name="warm_ps")
            for r in range(50):
                nc.tensor.matmul(
                    warm_ps[:, 0:P], warm[:], warm[:],
                    start=True, stop=True,
                )
            warm_h = persist.tile([P, 16], BF16, tag="warmh", name="warm_h")
            nc.scalar.activation(warm_h[:], warm[:, 0:16], gelu_fn)

            # x chunks: (tile, src column offset, h column offset, width).
            CHUNKS = [(xt[tcc], 0, tcc * 512, 512) for tcc in range(TC)]
            NCH = len(CHUNKS)

            def emit_l1(k, stat, after_group=None, order=None):
                """h[F,T] = gelu(W1.T @ x + b1); stat(dc, fc) -> stationary AP.

                dc innermost: each psum group completes in DC consecutive
                matmuls and its gelu issues immediately -- smooth ACT cadence.
                `order` (expert 0) sequences groups so each head DMA's
                deadline falls as late as possible.
                """
                ht = [
                    hp.tile([P, T], BF16, tag="h", name=f"h_{k}_{fc}")
                    for fc in range(FC)
                ]
                if order is None:
                    order = [
                        (fc, ch) for fc in range(FC) for ch in range(NCH)
                    ]
                for gidx, (fc, ch) in enumerate(order):
                    xtile, soff, hoff, width = CHUNKS[ch]
                    ph = psA.tile(
                        [P, 512], F32, tag="ph", name=f"ph_{k}_{fc}_{ch}"
                    )
                    for dc in range(DC):
                        nc.tensor.matmul(
                            ph[:, 0:width],
                            stat(dc, fc),
                            xtile[:, dc, soff:soff + width],
                            start=(dc == 0),
                            stop=(dc == DC - 1),
                        )
                    nc.scalar.activation(
                        ht[fc][:, hoff:hoff + width], ph[:, 0:width], gelu_fn,
                        bias=b1_sb[:, k * FC + fc:k * FC + fc + 1],
                    )
                    if after_group is not None:
                        after_group(gidx + 1, ph, width)
                return ht

            def emit_l2(k, ht, w2t):
                """eo[T,D] = h.T @ W2 ; acc (+)= eo * gw[:,k]; store when k==K-1."""
                for tt in range(TT):
                    po = psB.tile([P, 512], F32, tag="po", name=f"po_{k}_{tt}")
                    for fc in range(FC):
                        nc.tensor.matmul(
                            po[:, 0:D],
                            ht[fc][:, tt * P:(tt + 1) * P],
                            w2t[:, fc, :],
                            start=(fc == 0),
                            stop=(fc == FC - 1),
                        )
                    g = gw_sb[:, tt * K + k:tt * K + k + 1]
                    if k == K - 1 and K == 1:
                        o = ob.tile([P, D], BF16, tag="o", name=f"o_{tt}")
                        nc.vector.tensor_scalar_mul(o[:], po[:, 0:D], g)
                        eng = nc.sync if tt % 2 == 0 else nc.scalar
                        eng.dma_start(out[tt * P:(tt + 1) * P, :], o[:])
                    elif k == 0:
                        nc.vector.tensor_scalar_mul(acc[tt][:], po[:, 0:D], g)
                    elif k < K - 1:
                        nc.vector.scalar_tensor_tensor(
                            acc[tt][:], po[:, 0:D], g,
                            acc[tt][:], op0=mult, op1=add,
                        )
                    else:
                        o = ob.tile([P, D], BF16, tag="o", name=f"o_{tt}")
                        nc.vector.scalar_tensor_tensor(
                            o[:], po[:, 0:D], g,
                            acc[tt][:], op0=mult, op1=add,
                        )
                        eng = nc.sync if tt % 2 == 0 else nc.scalar
                        eng.dma_start(out[tt * P:(tt + 1) * P, :], o[:])

            # ---- weight tiles + prefetch scheduling ----
            # Queue-program placement does NOT throttle a DMA issue -- the
            # list scheduler hoists dependency-free DMAs to the queue head
            # (v4/v6/v8 lesson: early 2MB prefetches stole HBM bandwidth
            # from the critical head loads).  Gadget-copy WAW deps are the
            # reliable throttle.
            w2t = [
                w2p.tile([P, FC, D], BF16, tag="w2", name=f"w2_{k}")
                for k in range(K)
            ]
            w1t = [None] + [
                w1p.tile([P, DC, DF], BF16, tag="w1", name=f"w1_{k}")
                for k in range(1, K)
            ]

            def _head_feed(gidx, ph, width):
                # Gate the remaining head transfers on early psum groups:
                # a 2-element vector copy reads the group's psum and writes
                # a corner of the DMA destination, so the DMA's WAW dep
                # delays its transfer until the critical set is in use.
                if 2 <= gidx <= NG and gidx - 1 < NG:
                    g = gidx - 1
                    nc.vector.tensor_scalar_add(
                        w1g[g][0:1, 0, 0:2], ph[0:1, 0:2], 0.0)
                    nc.gpsimd.dma_start(w1g[g][:], w1z[g])
                elif gidx == NG + 1:
                    nc.vector.tensor_scalar_add(
                        gw_sb[0:1, 0:2], ph[0:1, 0:2], 0.0)
                    nc.gpsimd.dma_start(gw_sb[:], gwp[:])

            def _stat0(dc, fc):
                return w1g[fc // FG][:, fc % FG, dc * P:(dc + 1) * P]

            # fc-quadrant-major order: the first FG groups need only
            # w1g0 + xt0; each later input tile's deadline is a full
            # quadrant (~13.6us) out.
            order0 = [
                (fc, ch)
                for g in range(NG)
                for ch in range(NCH)
                for fc in range(g * FG, (g + 1) * FG)
            ]
            ht = emit_l1(0, _stat0, after_group=_head_feed, order=order0)

            # Next-expert weight streams on the sync ring (FIFO behind the
            # xt tiles), each gated by a gadget copy off an h tile whose
            # last write lands progressively deeper into expert-0 L1.
            nc.vector.tensor_scalar_add(
                w2t[0][0:1, 0, 0:2], ht[1][0:1, 0:2], 0.0)
            nc.sync.dma_start(w2t[0][:], w2s[0])
            if K > 1:
                nc.vector.tensor_scalar_add(
                    w1t[1][0:1, 0, 0:2], ht[5][0:1, 0:2], 0.0)
                nc.sync.dma_start(w1t[1][:], w1s[1])
                nc.vector.tensor_scalar_add(
                    w2t[1][0:1, 0, 0:2], ht[9][0:1, 0:2], 0.0)
                nc.sync.dma_start(w2t[1][:], w2s[1])
            if K > 2:
                nc.vector.tensor_scalar_add(
                    w1t[2][0:1, 0, 0:2], ht[13][0:1, 0:2], 0.0)
                nc.sync.dma_start(w1t[2][:], w1s[2])
            emit_l2(0, ht, w2t[0])

            for k in range(1, K):
                if k >= 2:
                    nc.gpsimd.dma_start(w2t[k][:], w2s[k])
                if k + 1 < K and k + 1 >= 3:
                    nc.gpsimd.dma_start(w1t[k + 1][:], w1s[k + 1])
                ht = emit_l1(
                    k,
                    lambda dc, fc, w=w1t[k]: w[:, dc, fc * P:(fc + 1) * P],
                )
                emit_l2(k, ht, w2t[k])

    _split_multi_waits(nc)
    return nc


# ---------------------------------------------------------------------------
# Host wrapper
# ---------------------------------------------------------------------------
_NC_CACHE: dict = {}


def _get_nc(K: int, T: int, D: int, DF: int):
    key = (K, T, D, DF)
    if key not in _NC_CACHE:
        _NC_CACHE[key] = build_moe_kernel(K, T, D, DF)
    return _NC_CACHE[key]


def _softmax(x, axis=-1):
    m = np.max(x, axis=axis, keepdims=True)
    e = np.exp(x - m)
    return e / np.sum(e, axis=axis, keepdims=True)


def run(inputs: dict, trace: bool = False, tmpdir: str | None = None):
    x = np.asarray(inputs["x"], dtype=np.float32)
    gate_w = np.asarray(inputs["gate_w"], dtype=np.float32)
    gate_b = np.asarray(inputs["gate_b"], dtype=np.float32)
    w1 = np.asarray(inputs["w1"], dtype=np.float32)
    b1 = np.asarray(inputs["b1"], dtype=np.float32)
    w2 = np.asarray(inputs["w2"], dtype=np.float32)
    b2 = np.asarray(inputs["b2"], dtype=np.float32)
    K = int(inputs["num_available"])

    B, S, D = x.shape
    DF = w1.shape[2]
    Ttot = B * S
    T = Ttot // N_CORES
    DC = D // P
    FC = DF // P
    TT = T // P
    TC = T // 512

    # Coarse routing on host (tiny): gate applied to the global token sum.
    ksum = x.sum(axis=(0, 1))
    coarse = gate_w @ ksum + gate_b
    idx = np.argsort(-coarse, kind="stable")[:K]

    gws = gate_w[idx]                      # [K, D]
    gbs = gate_b[idx]                      # [K]
    b1s = np.ascontiguousarray(b1[idx], dtype=np.float32)              # [K,DF]
    b2s = np.ascontiguousarray(b2[idx], dtype=np.float32)              # [K,D]

    # per-token softmax gating on host (0.02% of the FLOPs)
    xf = x.reshape(Ttot, D)
    logits = xf @ gws.T + gbs[None, :]                                 # [Ttot,K]
    gw = _softmax(logits, axis=1).astype(np.float32)

    # packed weights
    w1sel = np.ascontiguousarray(
        w1[idx].reshape(K, DC, P, DF).transpose(0, 2, 1, 3)
    ).astype(ml_dtypes.bfloat16)                                       # [K,P,DC,DF]
    w2sel = np.ascontiguousarray(
        w2[idx].reshape(K, FC, P, D).transpose(0, 2, 1, 3)
    ).astype(ml_dtypes.bfloat16)                                       # [K,P,FC,D]
    # expert-0 w1, group-major contiguous pack: [NG, P, FG, DC*128]
    w1z = np.ascontiguousarray(
        w1[idx[0]].reshape(DC, P, FC // 4, 4, P).transpose(2, 1, 3, 0, 4)
        .reshape(FC // 4, P, 4, DC * P)
    ).astype(ml_dtypes.bfloat16)
    b1pk = np.ascontiguousarray(
        b1s.reshape(K, FC, P).transpose(2, 0, 1).reshape(P, K * FC), dtype=np.float32
    )

    nc = _get_nc(K, T, D, DF)
    in_maps = []
    for c in range(N_CORES):
        gwc = gw[c * T:(c + 1) * T]  # [T, K]
        gwp = np.ascontiguousarray(
            gwc.reshape(TT, P, K).transpose(1, 0, 2).reshape(P, TT * K),
            dtype=np.float32,
        )
        # x packed chunk-major [TC, P, DC, 512]
        xc = xf[c * T:(c + 1) * T]
        xqc = np.ascontiguousarray(
            xc.reshape(TC, 512, DC, P).transpose(0, 3, 2, 1)
        ).astype(ml_dtypes.bfloat16)
        in_maps.append({
            "xq": xqc,
            "w1z": w1z,
            "w1s": w1sel,
            "w2s": w2sel,
            "gwp": gwp,
            "b1pk": b1pk,
        })

    res = run_bass_kernel_spmd(
        nc, in_maps, list(range(N_CORES)), trace=trace, tmpdir=tmpdir
    )
    outp = np.concatenate(
        [np.asarray(res.results[c]["out"]) for c in range(N_CORES)], axis=0
    ).astype(np.float32).reshape(B, S, D)

    # b2 contribution (zero in this problem's inputs; exact host-side fallback)
    if np.any(b2s):
        outp = outp + (gw @ b2s).reshape(B, S, D)

    return outp, res


def kernel(**inputs) -> np.ndarray:
    outp, _ = run(inputs, trace=False)
    return outp
